# revision 2
# baseline (speedup 1.0000x reference)
"""Pairwise squared-euclidean distance kernel for Trainium2 (8 NeuronCores).

z[i, j] = ||x_i||^2 + ||y_j||^2 - 2 * <x_i, y_j>

Sharding (v12): 2D 4x2 grid. x rows split across 4 row-groups (2048 each),
y columns split across 2 col-groups (4096 each). Each core computes a
[2048, 4096] tile of the output with no communication. Per-core HBM
traffic: in 6MB (x 2MB + yT 4MB) out 16.8MB (fp16).

Host prep inside kernel(): y is transposed once on the host (pure layout,
no FLOPs moved off-device) so each core DMA-loads yT [256, 4096] fp32
directly in the [d, j] layout the PE needs - this removes ALL y
transposes, casts and evacs that dominated v11's head and engine load.

Per-core algorithm:
  1. x loaded natural [128, 16, 256] fp32. ScalarE casts to bf16 with
     scale=-2 (exact pow2), PE transposes (identity matmul, 32 tiles),
     ScalarE evacs PSUM -> fp8e4 xT8 (ACT fp8-out is full rate; DVE
     fp8-out is ~3x slow - never use DVE for fp8 stores).
     xsq via ScalarE Square+accum from fp32 x (exact).
  2. yT chunks (4 x 1024 cols) loaded fp32 [128, 2, 1024]. ScalarE
     quantizes straight fp32 -> fp8 yT8 (one op per chunk). DVE computes
     yTsq[p,j] = yT0[p,j]^2 + yT1[p,j]^2 in bf16 (2 mult + 1 add).
     Chunks 2-3 stream behind the main loop.
  3. Main loop: 32 groups (2 y-halves x 16 m-tiles) of one 4-bank PSUM
     tile [128, 2048] fp32. Per group: 4 fp8 DoubleRow matmuls (both
     128-d halves of the contraction in one 512-cycle pass each) + 4
     bf16 ones-matmuls that accumulate ysq[j] into every partition
     (ones stationary x yTsq moving). ~1.76us/group on PE.
  4. Evac: ScalarE activation(psum + xsq bias) on cols 0:1024, DVE
     tensor_scalar_add on cols 1024:2048, both straight to fp16 (wide
     2-bank ops halve per-op fixed cost vs per-bank evacs). Host
     upcasts fp16 -> fp32.
  5. Output DMA [128, 2048] fp16 (512KB) per group on the sync HWDGE
     queue (issue cost ~0.6us lives on the idle Sync engine, not
     ScalarE).

Known-good environment notes (from v11): tensor_tensor_reduce crashes
the device - do not use. fp32 XBAR wedges the device; SWDGE cast-DMA
races its consumers - avoided (no XBAR, no cast-DMA at all here).
gpsimd compute interferes with DVE via SBUF ports - gpsimd left idle.
"""

import os

import numpy as np

import concourse.bacc as bacc
import concourse.mybir as mybir
import concourse.tile as tile
from concourse.bass_utils import run_bass_kernel_spmd
from concourse.masks import make_identity

N_CORES = 8
N_FULL = 8192  # total x rows
M_FULL = 8192  # total y rows
D = 256  # feature dim

R_SHARDS = 4  # x-row shards
C_SHARDS = 2  # y-col shards
N_SHARD = N_FULL // R_SHARDS  # 2048 x rows per core
M_SHARD = M_FULL // C_SHARDS  # 4096 y rows per core

P = 128
NT = 512  # one fp32 PSUM bank
GRP = 4  # PSUM banks per group
QCOLS = GRP * NT  # 2048
Q = M_SHARD // QCOLS  # 2 y halves
M_TILES = N_SHARD // P  # 16
YCH = 1024  # y cols per load chunk
N_YCH = M_SHARD // YCH  # 4
XCH_T = 8  # x m-tiles per load/cast chunk
EVAC_SPLIT = 1024  # cols 0:split -> ScalarE evac, split:2048 -> DVE

FP32 = mybir.dt.float32
BF16 = mybir.dt.bfloat16
FP16 = mybir.dt.float16
FP8 = mybir.dt.float8e4
DR = mybir.MatmulPerfMode.DoubleRow
AF = mybir.ActivationFunctionType
ALU = mybir.AluOpType

_CACHE = {}
LAST_RESULTS = None


def _build():
    nc = bacc.Bacc("TRN2", target_bir_lowering=False, debug=False, num_devices=N_CORES)
    x_d = nc.dram_tensor("x", [N_SHARD, D], FP32, kind="ExternalInput").ap()
    yt_d = nc.dram_tensor("yt", [D, M_SHARD], FP32, kind="ExternalInput").ap()
    out_d = nc.dram_tensor("out", [N_SHARD, M_SHARD], FP16, kind="ExternalOutput").ap()

    with tile.TileContext(nc) as tc:
        with (
            tc.tile_pool(name="const", bufs=1) as const,
            tc.tile_pool(name="sq", bufs=4) as sqp,
            tc.tile_pool(name="ystage", bufs=3) as ystage,
            tc.tile_pool(name="outp", bufs=4) as outp,
            tc.tile_pool(name="psmm", bufs=2, space="PSUM") as psmm,
        ):
            ones = const.tile([P, P], BF16)
            nc.vector.memset(ones[:], 1.0)
            identity = const.tile([P, P], BF16)
            make_identity(nc, identity)

            xsq = const.tile([P, M_TILES], FP32)
            x_nat = const.tile([P, M_TILES, D], FP32)
            xbf = const.tile([P, M_TILES, D], BF16)
            xT8 = const.tile([P, 2, N_SHARD], FP8, name="xT8")
            yT8 = const.tile([P, 2, M_SHARD], FP8, name="yT8")
            yTsq = const.tile([P, M_SHARD], BF16, name="yTsq")

            # ---- x pieces ----
            def x_load(xc):
                rows = slice(xc * XCH_T * P, (xc + 1) * XCH_T * P)
                nc.scalar.dma_start(
                    x_nat[:, xc * XCH_T : (xc + 1) * XCH_T, :],
                    x_d[rows, :].rearrange("(t p) d -> p t d", p=P),
                )

            def x_cast(xc):
                sl = slice(xc * XCH_T, (xc + 1) * XCH_T)
                nc.scalar.activation(
                    xbf[:, sl, :], x_nat[:, sl, :], AF.Identity, scale=-2.0
                )

            def x_tr(xc, c):
                # transpose 8 m-tiles of half c -> xT8 cols
                ps = psmm.tile([P, XCH_T * P], BF16, tag="mm", name=f"xtr{xc}{c}")
                for t in range(XCH_T):
                    nc.tensor.transpose(
                        ps[:, t * P : (t + 1) * P],
                        xbf[:, xc * XCH_T + t, c * P : (c + 1) * P],
                        identity,
                    )
                cols = slice(xc * XCH_T * P, (xc + 1) * XCH_T * P)
                nc.scalar.copy(xT8[:, c, cols], ps[:])

            def x_sq(t):
                sq = sqp.tile([P, D], FP32, tag="sq")
                nc.scalar.activation(
                    sq[:], x_nat[:, t, :], AF.Square, accum_out=xsq[:, t : t + 1]
                )

            # ---- y chunk staging ----
            ystg = {}

            def y_load(ch):
                cols = slice(ch * YCH, (ch + 1) * YCH)
                yst = ystage.tile([P, 2, YCH], FP32, tag="yst")
                nc.sync.dma_start(
                    yst[:], yt_d[:, cols].rearrange("(h p) c -> p h c", p=P)
                )
                ystg[ch] = yst

            def y_quant(ch):
                cols = slice(ch * YCH, (ch + 1) * YCH)
                nc.scalar.copy(yT8[:, :, cols], ystg[ch][:])

            def y_tsq(ch):
                cols = slice(ch * YCH, (ch + 1) * YCH)
                yst = ystg[ch]
                t0 = sqp.tile([P, YCH], BF16, tag="t0")
                nc.vector.tensor_tensor(t0[:], yst[:, 0, :], yst[:, 0, :], ALU.mult)
                t1 = sqp.tile([P, YCH], BF16, tag="t1")
                nc.vector.tensor_tensor(t1[:], yst[:, 1, :], yst[:, 1, :], ALU.mult)
                nc.vector.tensor_tensor(yTsq[:, cols], t0[:], t1[:], ALU.add)

            # ---- head: loads first, then prep. x fully prepped in head;
            # y chunks 0-1 prepped in head, 2-3 streamed behind main loop.
            y_load(0)
            x_load(0)
            x_load(1)
            y_load(1)
            x_cast(0)
            y_quant(0)
            x_cast(1)
            x_tr(0, 0)
            x_tr(0, 1)
            x_tr(1, 0)
            x_tr(1, 1)
            y_tsq(0)
            y_quant(1)
            y_tsq(1)
            for t in range(M_TILES):
                x_sq(t)

            # ---- main loop: per (y-half q, m-tile m): one 4-bank group
            for q in range(Q):
                for m in range(M_TILES):
                    if q == 0 and m == 0:
                        y_load(2)
                    if q == 0 and m == 2:
                        y_load(3)
                    if q == 0 and m == 5:
                        y_quant(2)
                        y_tsq(2)
                    if q == 0 and m == 9:
                        y_quant(3)
                        y_tsq(3)
                    lhs8 = xT8[:, :, m * P : (m + 1) * P]
                    pms = psmm.tile([P, QCOLS], FP32, tag="mm", name=f"pm_{q}_{m}")
                    for k in range(GRP):
                        n = q * GRP + k
                        nc.tensor.matmul(
                            pms[:, k * NT : (k + 1) * NT],
                            lhs8,
                            yT8[:, :, n * NT : (n + 1) * NT],
                            perf_mode=DR,
                            start=True,
                            stop=False,
                        )
                    for k in range(GRP):
                        n = q * GRP + k
                        nc.tensor.matmul(
                            pms[:, k * NT : (k + 1) * NT],
                            ones[:],
                            yTsq[:, n * NT : (n + 1) * NT],
                            start=False,
                            stop=True,
                        )
                    ot = outp.tile([P, QCOLS], FP16, tag="ot")
                    nc.scalar.activation(
                        ot[:, :EVAC_SPLIT],
                        pms[:, :EVAC_SPLIT],
                        AF.Identity,
                        bias=xsq[:, m : m + 1],
                        scale=1.0,
                    )
                    nc.vector.tensor_scalar_add(
                        ot[:, EVAC_SPLIT:], pms[:, EVAC_SPLIT:], xsq[:, m : m + 1]
                    )
                    nc.sync.dma_start(
                        out_d[m * P : (m + 1) * P, q * QCOLS : (q + 1) * QCOLS],
                        ot[:],
                    )

    nc.compile()
    return nc


def _get_nc():
    if "nc" not in _CACHE:
        _CACHE["nc"] = _build()
    return _CACHE["nc"]


def kernel(x: np.ndarray, y: np.ndarray) -> np.ndarray:
    global LAST_RESULTS
    x = np.ascontiguousarray(np.asarray(x, dtype=np.float32))
    y = np.ascontiguousarray(np.asarray(y, dtype=np.float32))
    assert x.shape == (N_FULL, D) and y.shape == (M_FULL, D)

    nc = _get_nc()
    yt = y.T  # [D, M_FULL], layout prep only
    yhalves = [
        np.ascontiguousarray(yt[:, c * M_SHARD : (c + 1) * M_SHARD])
        for c in range(C_SHARDS)
    ]
    in_maps = []
    for core in range(N_CORES):
        r, c = divmod(core, C_SHARDS)
        in_maps.append(
            {"x": x[r * N_SHARD : (r + 1) * N_SHARD], "yt": yhalves[c]}
        )
    res = run_bass_kernel_spmd(
        nc,
        in_maps,
        core_ids=list(range(N_CORES)),
        trace=bool(os.environ.get("BASS_KERNEL_TRACE")),
    )
    LAST_RESULTS = res
    out = np.empty((N_FULL, M_FULL), dtype=np.float32)
    for core in range(N_CORES):
        r, c = divmod(core, C_SHARDS)
        out[
            r * N_SHARD : (r + 1) * N_SHARD, c * M_SHARD : (c + 1) * M_SHARD
        ] = res.results[core]["out"].astype(np.float32)
    return out


# revision 3
# speedup vs baseline: 1.2474x; 1.2474x over previous
"""Pairwise squared-euclidean distance kernel for Trainium2 (8 NeuronCores).

z[i, j] = ||x_i||^2 + ||y_j||^2 - 2 * <x_i, y_j>

Sharding (v13): 2D 4x2 grid. x rows split across 4 row-groups (2048 each),
y columns split across 2 col-groups (4096 each). Each core computes a
[2048, 4096] tile of the output with no communication. Per-core HBM
traffic: in 6MB (x 2MB + yT 4MB) out 16.8MB (fp16).

Host prep inside kernel(): y is transposed once on the host (pure layout,
no FLOPs moved off-device) so each core DMA-loads yT [256, 4096] fp32
directly in the [d, j] layout the PE needs - no y transposes on device.

Per-core algorithm:
  1. x loaded natural in 4 chunks of 4 m-tiles (0.5MB DMAs on the ACT
     HWDGE ring, issued before any ScalarE compute). Chunks 0-1 cast to
     bf16*(-2) on ScalarE, chunks 2-3 on DVE (tensor_scalar mult). PE
     transposes (identity matmul), ScalarE evacs PSUM -> fp8e4 xT8
     (ACT fp8-out is full rate; DVE fp8-out is ~3x slow).
     xsq via ScalarE Square+accum from fp32 x, spread one m-tile per
     group inside the main loop (needed only at that group's evac).
  2. yT chunks (4 x 1024 cols) loaded fp32 [128, 2, 1024] on the sync
     ring. ScalarE quantizes fp32 -> fp8 yT8 (split per 128-d half so
     inserts stay under ~1us). DVE computes yTsq = yT0^2 + yT1^2 in
     bf16 (2 mult + 1 add). Chunks 2-3 stream behind the main loop.
  3. Main loop: 32 groups (2 y-halves x 16 m-tiles). Per group TWO
     2-bank PSUM tiles [128, 1024] fp32 - one evacuated by ScalarE,
     one by DVE, so the two evacs run in parallel (same-tile evacs
     serialize; different PSUM banks don't). Per group: 4 fp8
     DoubleRow matmuls (full 256-d contraction in one 512-cycle pass
     each) + 4 bf16 ones-matmuls accumulating ysq[j] (ones stationary
     x yTsq moving). ~1.9us/group on PE.
  4. Evac: ScalarE activation(psumA + xsq bias) -> fp16 ot[:, :1024],
     DVE tensor_scalar_add(psumB + xsq) -> fp16 ot[:, 1024:], in
     parallel. Host upcasts fp16 -> fp32.
  5. Output DMA [128, 2048] fp16 (512KB) per group on the sync HWDGE
     queue (issue cost ~0.6us lives on the idle Sync engine).

Known-good environment notes: tensor_tensor_reduce crashes the device.
fp32 XBAR wedges the device; SWDGE cast-DMA races its consumers - both
avoided (no XBAR, no cast-DMA). gpsimd compute interferes with DVE via
SBUF ports - gpsimd left idle. One shared PSUM tile across the two
evac engines serializes them via Tile deps - keep them on separate
tiles (v12 lesson, cost ~13us).
"""

import os

import numpy as np

import concourse.bacc as bacc
import concourse.mybir as mybir
import concourse.tile as tile
from concourse.bass_utils import run_bass_kernel_spmd
from concourse.masks import make_identity

N_CORES = 8
N_FULL = 8192  # total x rows
M_FULL = 8192  # total y rows
D = 256  # feature dim

R_SHARDS = 4  # x-row shards
C_SHARDS = 2  # y-col shards
N_SHARD = N_FULL // R_SHARDS  # 2048 x rows per core
M_SHARD = M_FULL // C_SHARDS  # 4096 y cols per core

P = 128
NT = 512  # one fp32 PSUM bank
GRP = 4  # PSUM banks per group
QCOLS = GRP * NT  # 2048
Q = M_SHARD // QCOLS  # 2 y halves
M_TILES = N_SHARD // P  # 16
YCH = 1024  # y cols per load chunk
N_YCH = M_SHARD // YCH  # 4
XCH_T = 4  # x m-tiles per load/cast chunk
N_XCH = M_TILES // XCH_T  # 4

FP32 = mybir.dt.float32
BF16 = mybir.dt.bfloat16
FP16 = mybir.dt.float16
FP8 = mybir.dt.float8e4
DR = mybir.MatmulPerfMode.DoubleRow
AF = mybir.ActivationFunctionType
ALU = mybir.AluOpType

_CACHE = {}
LAST_RESULTS = None


def _build():
    nc = bacc.Bacc("TRN2", target_bir_lowering=False, debug=False, num_devices=N_CORES)
    x_d = nc.dram_tensor("x", [N_SHARD, D], FP32, kind="ExternalInput").ap()
    yt_d = nc.dram_tensor("yt", [D, M_SHARD], FP32, kind="ExternalInput").ap()
    out_d = nc.dram_tensor("out", [N_SHARD, M_SHARD], FP16, kind="ExternalOutput").ap()

    with tile.TileContext(nc) as tc:
        with (
            tc.tile_pool(name="const", bufs=1) as const,
            tc.tile_pool(name="sq", bufs=4) as sqp,
            tc.tile_pool(name="ystage", bufs=3) as ystage,
            tc.tile_pool(name="outp", bufs=4) as outp,
            tc.tile_pool(name="psmm", bufs=4, space="PSUM") as psmm,
        ):
            ones = const.tile([P, P], BF16)
            nc.vector.memset(ones[:], 1.0)
            identity = const.tile([P, P], BF16)
            make_identity(nc, identity)

            xsq = const.tile([P, M_TILES], FP32)
            x_nat = const.tile([P, M_TILES, D], FP32)
            xbf = const.tile([P, M_TILES, D], BF16)
            xT8 = const.tile([P, 2, N_SHARD], FP8, name="xT8")
            yT8 = const.tile([P, 2, M_SHARD], FP8, name="yT8")
            yTsq = const.tile([P, M_SHARD], BF16, name="yTsq")

            # ---- x pieces ----
            def x_load(xc):
                rows = slice(xc * XCH_T * P, (xc + 1) * XCH_T * P)
                nc.scalar.dma_start(
                    x_nat[:, xc * XCH_T : (xc + 1) * XCH_T, :],
                    x_d[rows, :].rearrange("(t p) d -> p t d", p=P),
                )

            def x_cast(xc, eng):
                sl = slice(xc * XCH_T, (xc + 1) * XCH_T)
                if eng is nc.scalar:
                    nc.scalar.activation(
                        xbf[:, sl, :], x_nat[:, sl, :], AF.Identity, scale=-2.0
                    )
                else:
                    nc.vector.tensor_scalar_mul(xbf[:, sl, :], x_nat[:, sl, :], -2.0)

            def x_tr(xc, c):
                # transpose XCH_T m-tiles of half c -> xT8 cols
                ps = psmm.tile([P, XCH_T * P], BF16, tag="mm", name=f"xtr{xc}{c}")
                for t in range(XCH_T):
                    nc.tensor.transpose(
                        ps[:, t * P : (t + 1) * P],
                        xbf[:, xc * XCH_T + t, c * P : (c + 1) * P],
                        identity,
                    )
                cols = slice(xc * XCH_T * P, (xc + 1) * XCH_T * P)
                nc.scalar.copy(xT8[:, c, cols], ps[:])

            def x_sq(t):
                sq = sqp.tile([P, D], FP32, tag="sq")
                nc.scalar.activation(
                    sq[:], x_nat[:, t, :], AF.Square, accum_out=xsq[:, t : t + 1]
                )

            # ---- y chunk staging ----
            ystg = {}

            def y_load(ch):
                cols = slice(ch * YCH, (ch + 1) * YCH)
                yst = ystage.tile([P, 2, YCH], FP32, tag="yst")
                nc.sync.dma_start(
                    yst[:], yt_d[:, cols].rearrange("(h p) c -> p h c", p=P)
                )
                ystg[ch] = yst

            def y_quant(ch, h):
                cols = slice(ch * YCH, (ch + 1) * YCH)
                nc.scalar.copy(yT8[:, h, cols], ystg[ch][:, h, :])

            def y_tsq_mul(ch, h):
                yst = ystg[ch]
                t = sqp.tile([P, YCH], BF16, tag=f"t{h}")
                nc.vector.tensor_tensor(t[:], yst[:, h, :], yst[:, h, :], ALU.mult)
                return t

            _tsq_tmp = {}

            def y_tsq(ch, step):
                if step == 0:
                    _tsq_tmp[(ch, 0)] = y_tsq_mul(ch, 0)
                elif step == 1:
                    _tsq_tmp[(ch, 1)] = y_tsq_mul(ch, 1)
                else:
                    cols = slice(ch * YCH, (ch + 1) * YCH)
                    nc.vector.tensor_tensor(
                        yTsq[:, cols],
                        _tsq_tmp.pop((ch, 0))[:],
                        _tsq_tmp.pop((ch, 1))[:],
                        ALU.add,
                    )

            # ---- head ----
            # DMA issues first: x chunks on the ACT ring (before any ScalarE
            # compute), y chunks on the sync ring.
            y_load(0)
            for xc in range(N_XCH):
                x_load(xc)
            y_load(1)
            # x chunk 0 chain first (first matmul needs xT8 m0 + yT8 ch0)
            x_cast(0, nc.scalar)
            x_tr(0, 0)
            x_tr(0, 1)
            y_quant(0, 0)
            y_quant(0, 1)
            x_cast(1, nc.scalar)
            x_tr(1, 0)
            x_tr(1, 1)
            y_tsq(0, 0)
            y_tsq(0, 1)
            y_tsq(0, 2)
            y_quant(1, 0)
            y_quant(1, 1)
            x_cast(2, nc.vector)
            x_tr(2, 0)
            x_tr(2, 1)
            y_tsq(1, 0)
            y_tsq(1, 1)
            y_tsq(1, 2)
            x_cast(3, nc.vector)
            x_tr(3, 0)
            x_tr(3, 1)

            # ---- main loop ----
            for q in range(Q):
                for m in range(M_TILES):
                    if q == 0:
                        # stream y chunks 2-3 (needed from q==1) in small ops
                        if m == 0:
                            y_load(2)
                        if m == 2:
                            y_load(3)
                        if m == 4:
                            y_quant(2, 0)
                        if m == 5:
                            y_quant(2, 1)
                        if m in (6, 7, 8):
                            y_tsq(2, m - 6)
                        if m == 9:
                            y_quant(3, 0)
                        if m == 10:
                            y_quant(3, 1)
                        if m in (11, 12, 13):
                            y_tsq(3, m - 11)
                        x_sq(m)
                    lhs8 = xT8[:, :, m * P : (m + 1) * P]
                    pmA = psmm.tile([P, 2 * NT], FP32, tag="mm", name=f"pa_{q}_{m}")
                    pmB = psmm.tile([P, 2 * NT], FP32, tag="mm", name=f"pb_{q}_{m}")
                    for k in range(GRP):
                        n = q * GRP + k
                        pm = pmA if k < 2 else pmB
                        nc.tensor.matmul(
                            pm[:, (k % 2) * NT : (k % 2 + 1) * NT],
                            lhs8,
                            yT8[:, :, n * NT : (n + 1) * NT],
                            perf_mode=DR,
                            start=True,
                            stop=False,
                        )
                    for k in range(GRP):
                        n = q * GRP + k
                        pm = pmA if k < 2 else pmB
                        nc.tensor.matmul(
                            pm[:, (k % 2) * NT : (k % 2 + 1) * NT],
                            ones[:],
                            yTsq[:, n * NT : (n + 1) * NT],
                            start=False,
                            stop=True,
                        )
                    ot = outp.tile([P, QCOLS], FP16, tag="ot")
                    nc.scalar.activation(
                        ot[:, : 2 * NT],
                        pmA[:],
                        AF.Identity,
                        bias=xsq[:, m : m + 1],
                        scale=1.0,
                    )
                    nc.vector.tensor_scalar_add(
                        ot[:, 2 * NT :], pmB[:], xsq[:, m : m + 1]
                    )
                    nc.sync.dma_start(
                        out_d[m * P : (m + 1) * P, q * QCOLS : (q + 1) * QCOLS],
                        ot[:],
                    )

    nc.compile()
    return nc


def _get_nc():
    if "nc" not in _CACHE:
        _CACHE["nc"] = _build()
    return _CACHE["nc"]


def kernel(x: np.ndarray, y: np.ndarray) -> np.ndarray:
    global LAST_RESULTS
    x = np.ascontiguousarray(np.asarray(x, dtype=np.float32))
    y = np.ascontiguousarray(np.asarray(y, dtype=np.float32))
    assert x.shape == (N_FULL, D) and y.shape == (M_FULL, D)

    nc = _get_nc()
    yt = y.T  # [D, M_FULL], layout prep only
    yhalves = [
        np.ascontiguousarray(yt[:, c * M_SHARD : (c + 1) * M_SHARD])
        for c in range(C_SHARDS)
    ]
    in_maps = []
    for core in range(N_CORES):
        r, c = divmod(core, C_SHARDS)
        in_maps.append({"x": x[r * N_SHARD : (r + 1) * N_SHARD], "yt": yhalves[c]})
    res = run_bass_kernel_spmd(
        nc,
        in_maps,
        core_ids=list(range(N_CORES)),
        trace=bool(os.environ.get("BASS_KERNEL_TRACE")),
    )
    LAST_RESULTS = res
    out = np.empty((N_FULL, M_FULL), dtype=np.float32)
    for core in range(N_CORES):
        r, c = divmod(core, C_SHARDS)
        out[r * N_SHARD : (r + 1) * N_SHARD, c * M_SHARD : (c + 1) * M_SHARD] = (
            res.results[core]["out"].astype(np.float32)
        )
    return out
